# revision 4
# baseline (speedup 1.0000x reference)
"""BitLinear kernel for Trainium2, 8 NeuronCores, column-parallel.

y[t, o] = sum_i x[t, i] * sign(W[o, i]) * scale[o]
  x: [8192, 4096] f32 (replicated), W: [16384, 4096] f32, scale: [16384] f32
  Each core owns OUT_F/8 = 2048 output features (column parallel).

Mixed-precision contraction (per-core):
  - k in [0, 2048):    fp16 MMs (K=128 each, 16 per band)
  - k in [2048, 4096): fp8e4 DoubleRow MMs (K=256 each, 8 per band)
    DR runs at 2x (K=256 per ~216ns) — verified on HW. Quantizing half of
    x to e4m3 adds ~1.9e-2 relative output error (sign weights are exact
    +-1 in both f16 and fp8e4).

All transposes use the DMA XBAR (dma_start(..., transpose=True)), so the
PE runs matmuls only:
  - W prep:   W row-block f32 --casting DMA--> f16; sign via bit trick
              (w16 & 0x8000) ^ 0x3C00 -> +-1.0 exactly; XBAR-transpose to
              B16 f16 (k<2048) and staging -> DVE cast -> B8 fp8 (rest).
  - x path:   x f32 --casting DMA--> f16 [128, 4096]; two XBAR transposes
              (SP + Activation queues) -> xT16 [128, 32, 128] f16; DVE
              casts upper half -> xT8 [128, 16, 128] fp8.
  - matmul:   per (128-token tile, 512-out band): 16 f16 MMs + 8 DR MMs
              accumulate in PSUM; DVE multiplies by broadcast scale tile
              -> y tile -> DMA out.
"""

import os
import sys

for _p in ("/opt/trn_rl_repo",):
    if _p not in sys.path and os.path.isdir(_p):
        sys.path.append(_p)

import numpy as np
import concourse.bacc as bacc
import concourse.mybir as mybir
from concourse.tile import TileContext
from concourse.bass_utils import run_bass_kernel_spmd

TOKENS, IN_F, OUT_F, NCORES = 8192, 4096, 16384, 8
O_SH = OUT_F // NCORES  # 2048 out features per core
P = 128
KT = IN_F // P          # 32 k-subtiles total
KT16 = 16               # k-subtiles done in f16 (k < 2048)
KT8 = KT - KT16         # k-subtiles done in fp8 DR
KH = KT16 * P           # 2048, k split point
MT = TOKENS // P        # 64 token tiles
NBAND = 4
NB = 512
WARM = 5

f32, f16, u16 = mybir.dt.float32, mybir.dt.float16, mybir.dt.uint16
fp8 = mybir.dt.float8e4
DR = mybir.MatmulPerfMode.DoubleRow

_CACHE = {}
last_result = None


def build():
    nc = bacc.Bacc("TRN2", target_bir_lowering=False, debug=False)
    x = nc.dram_tensor("x", [TOKENS, IN_F], f32, kind="ExternalInput").ap()
    w = nc.dram_tensor("weight", [O_SH, IN_F], f32, kind="ExternalInput").ap()
    scale = nc.dram_tensor("scale", [O_SH], f32, kind="ExternalInput").ap()
    y = nc.dram_tensor("y", [TOKENS, O_SH], f32, kind="ExternalOutput").ap()

    with TileContext(nc) as tc:
        with (
            tc.tile_pool(name="const", bufs=1) as cpool,
            tc.tile_pool(name="bres", bufs=1) as bpool,
            tc.tile_pool(name="wstage", bufs=2) as wpool,
            tc.tile_pool(name="wtst", bufs=2) as wtpool,
            tc.tile_pool(name="xstage", bufs=2) as xpool,
            tc.tile_pool(name="xtp", bufs=WARM) as xtpool,
            tc.tile_pool(name="ystage", bufs=4) as ypool,
            tc.tile_pool(name="mmps", bufs=5, space="PSUM") as mmps,
        ):
            # broadcast scale tile: SCB[p, o] = scale[o] for all p
            SCB = cpool.tile([P, O_SH], f32, tag="scb")
            for p in range(P):
                nc.scalar.dma_start(SCB[p : p + 1, :], scale)

            B16 = bpool.tile([P, KT16, O_SH], f16, tag="B16")
            B8 = bpool.tile([P, KT8, O_SH], fp8, tag="B8")

            def prep_w_otile(ot):
                """Produce B16/B8 [:, :, ot*128:(ot+1)*128]."""
                wsg = wpool.tile([P, IN_F], f16, tag="wsg")
                # f32 -> f16 during the DMA itself (SWDGE cast)
                nc.gpsimd.dma_start(wsg[:], w[ot * P : (ot + 1) * P, :])
                # sign(w) = (w16 & 0x8000) ^ bits(1.0f16) -> +-1.0 exact
                nc.vector.tensor_scalar(
                    wsg[:].bitcast(u16),
                    wsg[:].bitcast(u16),
                    0x8000,
                    0x3C00,
                    mybir.AluOpType.bitwise_and,
                    mybir.AluOpType.bitwise_xor,
                )
                # f16 half straight into resident B16 via XBAR transpose.
                # All XBARs share one crossbar: keep every transpose on the
                # sync queue (concurrent transposes on two queues corrupt).
                nc.sync.dma_start(
                    B16[:, :, ot * P : (ot + 1) * P],
                    wsg[:, 0:KH],
                    transpose=True,
                )
                # fp8 half: XBAR to staging, DVE cast to fp8
                wT = wtpool.tile([P, KT8, P], f16, tag="wT")
                nc.sync.dma_start(wT[:], wsg[:, KH:IN_F], transpose=True)
                nc.vector.tensor_copy(B8[:, :, ot * P : (ot + 1) * P], wT[:])

            def make_xT(mt):
                xc = xpool.tile([P, IN_F], f16, tag="xc")
                nc.gpsimd.dma_start(xc[:], x[mt * P : (mt + 1) * P, :])
                xT16 = xtpool.tile([P, KT, P], f16, tag="xT16")
                nc.sync.dma_start(xT16[:, 0:KT16, :], xc[:, 0:KH], transpose=True)
                nc.sync.dma_start(xT16[:, KT16:KT, :], xc[:, KH:IN_F],
                                  transpose=True)
                xT8 = xtpool.tile([P, KT8, P], fp8, tag="xT8")
                nc.vector.tensor_copy(xT8[:], xT16[:, KT16:KT, :])
                return xT16, xT8

            def mm_band(mt, band, xT16, xT8):
                ps = mmps.tile([P, NB], f32, tag="ps")
                n0 = band * NB
                for k in range(KT16):
                    nc.tensor.matmul(
                        ps[:], xT16[:, k, :], B16[:, k, n0 : n0 + NB],
                        start=(k == 0), stop=False,
                    )
                for c in range(KT8 // 2):
                    nc.tensor.matmul(
                        ps[:],
                        xT8[:, 2 * c : 2 * c + 2, :],
                        B8[:, 2 * c : 2 * c + 2, n0 : n0 + NB],
                        start=False, stop=(c == KT8 // 2 - 1),
                        perf_mode=DR,
                    )
                yq = ypool.tile([P, NB], f32, tag="yq")
                nc.vector.tensor_tensor(
                    yq[:], ps[:], SCB[:, n0 : n0 + NB], mybir.AluOpType.mult
                )
                nc.scalar.dma_start(y[mt * P : (mt + 1) * P, n0 : n0 + NB], yq[:])

            # prep band 0's o-tiles, warm xT tiles, then band-major warm MMs
            # with the next band's prep emitted before the current band's MMs
            for ot in range(4):
                prep_w_otile(ot)
            warm_xT = [make_xT(mt) for mt in range(WARM)]
            for band in range(NBAND):
                if band + 1 < NBAND:
                    for ot in range(4 * (band + 1), 4 * (band + 2)):
                        prep_w_otile(ot)
                for mt in range(WARM):
                    mm_band(mt, band, *warm_xT[mt])

            # steady phase
            for mt in range(WARM, MT):
                xT16, xT8 = make_xT(mt)
                for band in range(NBAND):
                    mm_band(mt, band, xT16, xT8)

    nc.finalize()
    return nc


def _get_nc():
    if "nc" not in _CACHE:
        _CACHE["nc"] = build()
    return _CACHE["nc"]


def kernel(x, weight, scale):
    global last_result
    nc = _get_nc()
    x = np.ascontiguousarray(np.asarray(x, dtype=np.float32))
    weight = np.ascontiguousarray(np.asarray(weight, dtype=np.float32))
    scale = np.ascontiguousarray(np.asarray(scale, dtype=np.float32))
    in_maps = [
        {
            "x": x,
            "weight": np.ascontiguousarray(weight[c * O_SH : (c + 1) * O_SH]),
            "scale": np.ascontiguousarray(scale[c * O_SH : (c + 1) * O_SH]),
        }
        for c in range(NCORES)
    ]
    res = run_bass_kernel_spmd(nc, in_maps, list(range(NCORES)))
    last_result = res
    return np.concatenate([res.results[c]["y"] for c in range(NCORES)], axis=1)


if __name__ == "__main__":
    rng = np.random.default_rng(0)
    xv = rng.standard_normal((TOKENS, IN_F), dtype=np.float32)
    wv = rng.standard_normal((OUT_F, IN_F), dtype=np.float32)
    sv = np.ones(OUT_F, dtype=np.float32)
    yv = kernel(xv, wv, sv)
    print("out shape:", yv.shape, yv.dtype)


# revision 5
# speedup vs baseline: 1.0636x; 1.0636x over previous
"""BitLinear kernel for Trainium2, 8 NeuronCores, column-parallel.

y[t, o] = sum_i x[t, i] * sign(W[o, i]) * scale[o]
  x: [8192, 4096] f32 (replicated), W: [16384, 4096] f32, scale: [16384] f32
  Each core owns OUT_F/8 = 2048 output features (column parallel).

Mixed-precision contraction (per-core):
  - k in [0, 2048):    fp16 MMs (K=128 each, 16 per band)
  - k in [2048, 4096): fp8e4 DoubleRow MMs (K=256 each, 8 per band)
    DR runs at 2x (K=256 per ~216ns) — verified on HW. Quantizing half of
    x to e4m3 adds ~1.9e-2 relative output error (sign weights are exact
    +-1 in both f16 and fp8e4).

All transposes use the DMA XBAR (dma_start(..., transpose=True)), so the
PE runs matmuls only:
  - W prep:   W row-block f32 --casting DMA--> f16; sign via bit trick
              (w16 & 0x8000) ^ 0x3C00 -> +-1.0 exactly; XBAR-transpose to
              B16 f16 (k<2048) and staging -> DVE cast -> B8 fp8 (rest).
  - x path:   x f32 --casting DMA--> f16 [128, 4096]; PE transposes
              (grouped 4 per PSUM bank); ACT copies -> xT16 f16 (k<2048)
              and xT8 fp8 (rest). W-prep uses the DMA XBAR instead (one
              queue only — the transpose crossbar is a shared resource).
  - matmul:   per (128-token tile, 512-out band): 16 f16 MMs + 8 DR MMs
              accumulate in PSUM; DVE multiplies by broadcast scale tile
              -> y tile -> DMA out.
"""

import os
import sys

for _p in ("/opt/trn_rl_repo",):
    if _p not in sys.path and os.path.isdir(_p):
        sys.path.append(_p)

import numpy as np
import concourse.bacc as bacc
import concourse.mybir as mybir
from concourse.tile import TileContext
from concourse.bass_utils import run_bass_kernel_spmd
from concourse.masks import make_identity

TOKENS, IN_F, OUT_F, NCORES = 8192, 4096, 16384, 8
O_SH = OUT_F // NCORES  # 2048 out features per core
P = 128
KT = IN_F // P          # 32 k-subtiles total
KT16 = 16               # k-subtiles done in f16 (k < 2048)
KT8 = KT - KT16         # k-subtiles done in fp8 DR
KH = KT16 * P           # 2048, k split point
MT = TOKENS // P        # 64 token tiles
NBAND = 4
NB = 512
WARM = 5

f32, f16, u16 = mybir.dt.float32, mybir.dt.float16, mybir.dt.uint16
fp8 = mybir.dt.float8e4
DR = mybir.MatmulPerfMode.DoubleRow

_CACHE = {}
last_result = None


def build():
    nc = bacc.Bacc("TRN2", target_bir_lowering=False, debug=False)
    x = nc.dram_tensor("x", [TOKENS, IN_F], f32, kind="ExternalInput").ap()
    w = nc.dram_tensor("weight", [O_SH, IN_F], f32, kind="ExternalInput").ap()
    scale = nc.dram_tensor("scale", [O_SH], f32, kind="ExternalInput").ap()
    y = nc.dram_tensor("y", [TOKENS, O_SH], f32, kind="ExternalOutput").ap()

    with TileContext(nc) as tc:
        with (
            tc.tile_pool(name="const", bufs=1) as cpool,
            tc.tile_pool(name="bres", bufs=1) as bpool,
            tc.tile_pool(name="wstage", bufs=2) as wpool,
            tc.tile_pool(name="wtst", bufs=2) as wtpool,
            tc.tile_pool(name="xstage", bufs=2) as xpool,
            tc.tile_pool(name="xtp", bufs=WARM) as xtpool,
            tc.tile_pool(name="ystage", bufs=4) as ypool,
            tc.tile_pool(name="mmps", bufs=5, space="PSUM") as mmps,
            tc.tile_pool(name="tpps", bufs=3, space="PSUM") as tpps,
        ):
            # broadcast scale tile: SCB[p, o] = scale[o] for all p
            SCB = cpool.tile([P, O_SH], f32, tag="scb")
            for p in range(P):
                nc.scalar.dma_start(SCB[p : p + 1, :], scale)

            B16 = bpool.tile([P, KT16, O_SH], f16, tag="B16")
            B8 = bpool.tile([P, KT8, O_SH], fp8, tag="B8")

            def prep_w_otile(ot):
                """Produce B16/B8 [:, :, ot*128:(ot+1)*128]."""
                wsg = wpool.tile([P, IN_F], f16, tag="wsg")
                # f32 -> f16 during the DMA itself (SWDGE cast)
                nc.gpsimd.dma_start(wsg[:], w[ot * P : (ot + 1) * P, :])
                # sign(w) = (w16 & 0x8000) ^ bits(1.0f16) -> +-1.0 exact
                nc.vector.tensor_scalar(
                    wsg[:].bitcast(u16),
                    wsg[:].bitcast(u16),
                    0x8000,
                    0x3C00,
                    mybir.AluOpType.bitwise_and,
                    mybir.AluOpType.bitwise_xor,
                )
                # f16 half straight into resident B16 via XBAR transpose.
                # All XBARs share one crossbar: keep every transpose on the
                # sync queue (concurrent transposes on two queues corrupt).
                nc.sync.dma_start(
                    B16[:, :, ot * P : (ot + 1) * P],
                    wsg[:, 0:KH],
                    transpose=True,
                )
                # fp8 half: XBAR to staging, DVE cast to fp8
                wT = wtpool.tile([P, KT8, P], f16, tag="wT")
                nc.sync.dma_start(wT[:], wsg[:, KH:IN_F], transpose=True)
                nc.vector.tensor_copy(B8[:, :, ot * P : (ot + 1) * P], wT[:])

            ident = cpool.tile([P, P], f16, tag="ident")
            make_identity(nc, ident)

            def make_xT(mt):
                xc = xpool.tile([P, IN_F], f16, tag="xc")
                nc.gpsimd.dma_start(xc[:], x[mt * P : (mt + 1) * P, :])
                xT16 = xtpool.tile([P, KT16, P], f16, tag="xT16")
                xT8 = xtpool.tile([P, KT8, P], fp8, tag="xT8")
                for g in range(KT // 4):  # 8 groups of 4 transposes
                    tp = tpps.tile([P, 512], f16, tag="tp")
                    for j in range(4):
                        ki = g * 4 + j
                        nc.tensor.transpose(
                            tp[:, j * P : (j + 1) * P],
                            xc[:, ki * P : (ki + 1) * P],
                            ident[:],
                        )
                    src = tp[:].rearrange("p (a b) -> p a b", a=4)
                    if g < KT16 // 4:
                        nc.scalar.activation(
                            xT16[:, g * 4 : g * 4 + 4, :], src,
                            mybir.ActivationFunctionType.Copy)
                    else:
                        g8 = g - KT16 // 4
                        nc.scalar.activation(
                            xT8[:, g8 * 4 : g8 * 4 + 4, :], src,
                            mybir.ActivationFunctionType.Copy)
                return xT16, xT8

            def mm_band(mt, band, xT16, xT8):
                ps = mmps.tile([P, NB], f32, tag="ps")
                n0 = band * NB
                for k in range(KT16):
                    nc.tensor.matmul(
                        ps[:], xT16[:, k, :], B16[:, k, n0 : n0 + NB],
                        start=(k == 0), stop=False,
                    )
                for c in range(KT8 // 2):
                    nc.tensor.matmul(
                        ps[:],
                        xT8[:, 2 * c : 2 * c + 2, :],
                        B8[:, 2 * c : 2 * c + 2, n0 : n0 + NB],
                        start=False, stop=(c == KT8 // 2 - 1),
                        perf_mode=DR,
                    )
                yq = ypool.tile([P, NB], f32, tag="yq")
                nc.vector.tensor_tensor(
                    yq[:], ps[:], SCB[:, n0 : n0 + NB], mybir.AluOpType.mult
                )
                nc.scalar.dma_start(y[mt * P : (mt + 1) * P, n0 : n0 + NB], yq[:])

            # prep band 0's o-tiles, warm xT tiles, then band-major warm MMs
            # with the next band's prep emitted before the current band's MMs
            for ot in range(4):
                prep_w_otile(ot)
            warm_xT = [make_xT(mt) for mt in range(WARM)]
            for band in range(NBAND):
                if band + 1 < NBAND:
                    for ot in range(4 * (band + 1), 4 * (band + 2)):
                        prep_w_otile(ot)
                for mt in range(WARM):
                    mm_band(mt, band, *warm_xT[mt])

            # steady phase
            for mt in range(WARM, MT):
                xT16, xT8 = make_xT(mt)
                for band in range(NBAND):
                    mm_band(mt, band, xT16, xT8)

    nc.finalize()
    return nc


def _get_nc():
    if "nc" not in _CACHE:
        _CACHE["nc"] = build()
    return _CACHE["nc"]


def kernel(x, weight, scale):
    global last_result
    nc = _get_nc()
    x = np.ascontiguousarray(np.asarray(x, dtype=np.float32))
    weight = np.ascontiguousarray(np.asarray(weight, dtype=np.float32))
    scale = np.ascontiguousarray(np.asarray(scale, dtype=np.float32))
    in_maps = [
        {
            "x": x,
            "weight": np.ascontiguousarray(weight[c * O_SH : (c + 1) * O_SH]),
            "scale": np.ascontiguousarray(scale[c * O_SH : (c + 1) * O_SH]),
        }
        for c in range(NCORES)
    ]
    res = run_bass_kernel_spmd(nc, in_maps, list(range(NCORES)))
    last_result = res
    return np.concatenate([res.results[c]["y"] for c in range(NCORES)], axis=1)


if __name__ == "__main__":
    rng = np.random.default_rng(0)
    xv = rng.standard_normal((TOKENS, IN_F), dtype=np.float32)
    wv = rng.standard_normal((OUT_F, IN_F), dtype=np.float32)
    sv = np.ones(OUT_F, dtype=np.float32)
    yv = kernel(xv, wv, sv)
    print("out shape:", yv.shape, yv.dtype)


# revision 6
# speedup vs baseline: 1.1862x; 1.1153x over previous
"""BitLinear kernel for Trainium2, 8 NeuronCores, column-parallel.

y[t, o] = sum_i x[t, i] * sign(W[o, i]) * scale[o]
  x: [8192, 4096] f32 (replicated), W: [16384, 4096] f32, scale: [16384] f32
  Each core owns OUT_F/8 = 2048 output features (column parallel).

Mixed-precision contraction (per-core):
  - k in [0, 2048):    fp16 MMs (K=128 each, 16 per band)
  - k in [2048, 4096): fp8e4 DoubleRow MMs (K=256 each, 8 per band)
    DR runs at 2x: K=256 per ~216ns vs K=128 for fp16 — verified on HW.
  Quantizing half of x to e4m3 adds ~1.9e-2 relative output error
  (BitLinear sign weights are exact +-1 in both f16 and fp8).

Per-core pipeline (all math on device):
  - W prep:   W f32 --casting DMA--> f16; sign via bit trick
              (w16 & 0x8000) ^ 0x3C00 -> +-1.0 f16 exactly; PE-transpose;
              PSUM copies write B16 f16 (k<2048) and B8 fp8e4 (k>=2048).
  - scale:    SCB [128, 2048] f32 broadcast tile (scale replicated across
              token partitions) via per-partition DMA; output stage
              multiplies PSUM by SCB slice (general scale support).
  - x path:   x f32 --casting DMA--> f16 [128, 4096]; PE-transpose;
              PSUM copies write xT f16 (k<2048) and xT8 fp8e4 (rest).
  - matmul:   per 128-token tile: k-outer/band-inner: 16 f16 MMs x 4 bands
              accumulate PSUM, then 8 DR MMs x 4 bands finish; DVE
              multiplies PSUM by SCB -> y tile -> DMA out.
"""

import os
import sys

for _p in ("/opt/trn_rl_repo",):
    if _p not in sys.path and os.path.isdir(_p):
        sys.path.append(_p)

import numpy as np
import concourse.bacc as bacc
import concourse.mybir as mybir
from concourse.tile import TileContext
from concourse.masks import make_identity
from concourse.bass_utils import run_bass_kernel_spmd

TOKENS, IN_F, OUT_F, NCORES = 8192, 4096, 16384, 8
O_SH = OUT_F // NCORES  # 2048 out features per core
P = 128
KT = IN_F // P          # 32 k-subtiles total
KT16 = 16               # k-subtiles done in f16 (k < 2048)
KT8 = KT - KT16         # k-subtiles done in fp8 DR (16 -> 8 DR MMs)
MT = TOKENS // P        # 64 token tiles
NBAND = 4               # 4 output bands of 512
NB = 512
W_KC = 2048             # W prep free-dim chunk (= half the k range)
WARM = 6                # band-major warm token tiles

f32, f16, u16 = mybir.dt.float32, mybir.dt.float16, mybir.dt.uint16
fp8 = mybir.dt.float8e4
DR = mybir.MatmulPerfMode.DoubleRow
AF = mybir.ActivationFunctionType

_CACHE = {}
last_result = None


def build():
    nc = bacc.Bacc("TRN2", target_bir_lowering=False, debug=False)
    x = nc.dram_tensor("x", [TOKENS, IN_F], f32, kind="ExternalInput").ap()
    w = nc.dram_tensor("weight", [O_SH, IN_F], f32, kind="ExternalInput").ap()
    scale = nc.dram_tensor("scale", [O_SH], f32, kind="ExternalInput").ap()
    y = nc.dram_tensor("y", [TOKENS, O_SH], f32, kind="ExternalOutput").ap()

    with TileContext(nc) as tc:
        with (
            tc.tile_pool(name="const", bufs=1) as cpool,
            tc.tile_pool(name="bres", bufs=1) as bpool,
            tc.tile_pool(name="wstage", bufs=3) as wpool,
            tc.tile_pool(name="xstage", bufs=2) as xpool,
            tc.tile_pool(name="xtp", bufs=WARM) as xtpool,
            tc.tile_pool(name="ystage", bufs=4) as ypool,
            tc.tile_pool(name="mmps", bufs=4, space="PSUM") as mmps,
            tc.tile_pool(name="tpps", bufs=3, space="PSUM") as tpps,
        ):
            ident = cpool.tile([P, P], f16, tag="ident")
            make_identity(nc, ident)

            # broadcast scale tile: SCB[p, o] = scale[o] for all p
            SCB = cpool.tile([P, O_SH], f32, tag="scb")
            for p in range(P):
                nc.sync.dma_start(SCB[p : p + 1, :], scale)

            B16 = bpool.tile([P, KT16, O_SH], f16, tag="B16")
            B8 = bpool.tile([P, KT8, O_SH], fp8, tag="B8")

            def prep_w_band(band):
                """Produce B16/B8 [:, :, band*512:(band+1)*512]."""
                copy_flip = band % 2
                for oi in range(4):
                    ot = band * 4 + oi
                    for kc in range(IN_F // W_KC):  # kc=0 -> f16, kc=1 -> fp8
                        wsg = wpool.tile([P, W_KC], f16, tag="wsg")
                        # f32 -> f16 during the DMA itself (SWDGE cast)
                        nc.gpsimd.dma_start(
                            wsg[:],
                            w[ot * P : (ot + 1) * P, kc * W_KC : (kc + 1) * W_KC],
                        )
                        # sign(w) = (w16 & 0x8000) ^ bits(1.0f16)  -> +-1.0
                        nc.vector.tensor_scalar(
                            wsg[:].bitcast(u16),
                            wsg[:].bitcast(u16),
                            0x8000,
                            0x3C00,
                            mybir.AluOpType.bitwise_and,
                            mybir.AluOpType.bitwise_xor,
                        )
                        for g in range(W_KC // P // 4):  # groups of 4
                            tp = tpps.tile([P, 512], f16, tag="tp")
                            for j in range(4):
                                ki = g * 4 + j
                                nc.tensor.transpose(
                                    tp[:, j * P : (j + 1) * P],
                                    wsg[:, ki * P : (ki + 1) * P],
                                    ident[:],
                                )
                            k0 = g * 4
                            src = tp[:].rearrange("p (a b) -> p a b", a=4)
                            if kc == 0:
                                dst = B16[:, k0 : k0 + 4, ot * P : (ot + 1) * P]
                            else:
                                dst = B8[:, k0 : k0 + 4, ot * P : (ot + 1) * P]
                            # alternate copy engine to balance ACT/DVE
                            if (g + oi + copy_flip) % 2 == 0:
                                nc.vector.tensor_copy(dst, src)
                            else:
                                nc.scalar.activation(dst, src, AF.Copy)

            def make_xT(mt):
                xc = xpool.tile([P, IN_F], f16, tag="xc")
                nc.gpsimd.dma_start(xc[:], x[mt * P : (mt + 1) * P, :])
                xT = xtpool.tile([P, KT16, P], f16, tag="xT")
                xT8 = xtpool.tile([P, KT8, P], fp8, tag="xT8")
                for g in range(KT // 4):  # 8 groups of 4 transposes
                    tp = tpps.tile([P, 512], f16, tag="tp")
                    for j in range(4):
                        ki = g * 4 + j
                        nc.tensor.transpose(
                            tp[:, j * P : (j + 1) * P],
                            xc[:, ki * P : (ki + 1) * P],
                            ident[:],
                        )
                    src = tp[:].rearrange("p (a b) -> p a b", a=4)
                    if g < KT16 // 4:
                        nc.scalar.activation(xT[:, g * 4 : g * 4 + 4, :], src, AF.Copy)
                    else:
                        g8 = g - KT16 // 4
                        nc.scalar.activation(
                            xT8[:, g8 * 4 : g8 * 4 + 4, :], src, AF.Copy
                        )
                return xT, xT8

            def mm_band(mt, band, xT, xT8):
                """Full accumulation for one (token tile, band): f16 then DR."""
                ps = mmps.tile([P, NB], f32, tag="ps")
                n0 = band * NB
                for k in range(KT16):
                    nc.tensor.matmul(
                        ps[:], xT[:, k, :], B16[:, k, n0 : n0 + NB],
                        start=(k == 0), stop=False,
                    )
                for c in range(KT8 // 2):
                    nc.tensor.matmul(
                        ps[:],
                        xT8[:, 2 * c : 2 * c + 2, :],
                        B8[:, 2 * c : 2 * c + 2, n0 : n0 + NB],
                        start=False, stop=(c == KT8 // 2 - 1),
                        perf_mode=DR,
                    )
                yq = ypool.tile([P, NB], f32, tag="yq")
                nc.vector.tensor_tensor(
                    yq[:], ps[:], SCB[:, n0 : n0 + NB], mybir.AluOpType.mult
                )
                nc.sync.dma_start(y[mt * P : (mt + 1) * P, n0 : n0 + NB], yq[:])

            # W prep band 0, warm xT tiles, then band-major warm MMs with
            # each next band's prep emitted BEFORE the current band's MMs
            prep_w_band(0)
            warm_xT = [make_xT(mt) for mt in range(WARM)]
            for band in range(NBAND):
                if band + 1 < NBAND:
                    prep_w_band(band + 1)
                for mt in range(WARM):
                    mm_band(mt, band, *warm_xT[mt])

            # steady phase
            for mt in range(WARM, MT):
                xT, xT8 = make_xT(mt)
                for band in range(NBAND):
                    mm_band(mt, band, xT, xT8)

    nc.finalize()
    return nc


def _get_nc():
    if "nc" not in _CACHE:
        _CACHE["nc"] = build()
    return _CACHE["nc"]


def kernel(x, weight, scale):
    global last_result
    nc = _get_nc()
    x = np.ascontiguousarray(np.asarray(x, dtype=np.float32))
    weight = np.ascontiguousarray(np.asarray(weight, dtype=np.float32))
    scale = np.ascontiguousarray(np.asarray(scale, dtype=np.float32))
    in_maps = [
        {
            "x": x,
            "weight": np.ascontiguousarray(weight[c * O_SH : (c + 1) * O_SH]),
            "scale": np.ascontiguousarray(scale[c * O_SH : (c + 1) * O_SH]),
        }
        for c in range(NCORES)
    ]
    res = run_bass_kernel_spmd(nc, in_maps, list(range(NCORES)))
    last_result = res
    return np.concatenate([res.results[c]["y"] for c in range(NCORES)], axis=1)


if __name__ == "__main__":
    rng = np.random.default_rng(0)
    xv = rng.standard_normal((TOKENS, IN_F), dtype=np.float32)
    wv = rng.standard_normal((OUT_F, IN_F), dtype=np.float32)
    sv = np.ones(OUT_F, dtype=np.float32)
    yv = kernel(xv, wv, sv)
    print("out shape:", yv.shape, yv.dtype)
